# revision 13
# baseline (speedup 1.0000x reference)
"""Two-layer GraphSAGE (mean aggregation) on 8 Trainium2 NeuronCores.

Strategy (dst-partitioned, graph/data parallel):
- Nodes partitioned by destination across 8 cores (12500 each, padded to
  12544 = 98*128). x replicated per core in bf16. The padded global row
  space is [half, core, 6272] so the between-layer AllGather can be split
  into two half-collectives whose outputs are exactly quarters 0-1 / 2-3 of
  the gather table (quarter = 25088 rows < int16 range).
- Edges bucketed per core by (group of 14 dst tiles, src quarter), sorted by
  dst tile. Source rows are fetched with large SWDGE dma_gather calls; the
  4 quarter calls of each block run on 4 SWDGE queues so Q7 descriptor
  generation proceeds in parallel. Slot layout is identical on every core
  (capacity = max count over cores; pad slots gather row 0 with a zero
  indicator) keeping the SPMD program uniform.
- Aggregation: per (column, dst tile) incidence, one DVE tensor_scalar
  builds a 0/1 indicator (iota == edst) and one PE matmul accumulates raw
  sums [feat, dst] into PSUM. Mean normalization is a per-partition scalar
  multiply after the W_l matmul.
- h tile = relu((sums^T @ W_l) * recip + bias + x_selfT @ W_r); self term
  loaded pre-transposed via HWDGE dma_start_transpose (bf16). The layer-2
  gathers for quarters 0-1 only wait on the first half-AllGather.

kernel(**inputs) -> np.ndarray takes FULL inputs, returns FULL [100000, 128]
float32 output; all sharding happens inside.
"""

import numpy as np

P = 128
NCORES = 8
NPC = 12500
TPC = 98
NPC_PAD = TPC * P            # 12544
NALL = NCORES * NPC_PAD      # 100352
NQ = 4
QROWS = NALL // NQ           # 25088
HROWS = NPC_PAD // 2         # 6272 rows per half per core (= 49 tiles)
TG = 14                      # dst tiles per group
NG = TPC // TG               # 7 groups
GB = 1                       # groups per gather block
NB = -(-NG // GB)            # 4 gather blocks


def _prep(edge_index):
    src = edge_index[0].astype(np.int64)
    dst = edge_index[1].astype(np.int64)
    core = dst // NPC
    loc = dst % NPC
    tl = loc // P
    off = loc % P
    g = tl // TG
    tl_loc = tl % TG
    # padded global row space: [half, core, HROWS]
    sloc = src % NPC
    half = sloc // HROWS
    srcpad = half * (NCORES * HROWS) + (src // NPC) * HROWS + sloc % HROWS
    q = srcpad // QROWS
    qrow = srcpad % QROWS

    key = ((core * NG + g) * NQ + q) * TG + tl_loc
    cnt = np.bincount(key, minlength=NCORES * NG * NQ * TG).reshape(
        NCORES, NG, NQ, TG
    )
    cap = cnt.max(axis=0)
    scum = np.zeros((NG, NQ, TG + 1), np.int64)
    np.cumsum(cap, axis=2, out=scum[:, :, 1:])
    segcols = -(-scum[:, :, TG] // P)

    blocks = [list(range(b * GB, min((b + 1) * GB, NG))) for b in range(NB)]
    colbase = np.zeros((NG, NQ), np.int64)
    calls = []                                  # (b, q, col0, callcols)
    ncols = 0
    for b, gs in enumerate(blocks):
        for qq in range(NQ):
            c0 = ncols
            for gg in gs:
                colbase[gg, qq] = ncols
                ncols += int(segcols[gg, qq])
            calls.append((b, qq, c0, ncols - c0))

    order = np.lexsort((tl_loc, q, g, core))
    sk = key[order]
    first = np.r_[True, sk[1:] != sk[:-1]]
    idx_of_first = np.where(first)[0]
    grp_id = np.cumsum(first) - 1
    rank = np.arange(len(sk)) - idx_of_first[grp_id]
    go, qo, to, co = g[order], q[order], tl_loc[order], core[order]
    slot = scum[go, qo, to] + rank
    gcol = colbase[go, qo] + slot // P
    prow = slot % P

    edst = np.full((NCORES, P, ncols), -1.0, np.float32)
    cnt_dst = np.bincount(dst, minlength=NCORES * NPC).astype(np.float64)
    recip_dst = (1.0 / np.maximum(cnt_dst, 1.0)).astype(np.float32)
    edst[co, prow, gcol] = (to * P + off[order]).astype(np.float32)
    recip_t = np.zeros((NCORES, P, TPC), np.float32)
    for c in range(NCORES):
        r = np.zeros(NPC_PAD, np.float32)
        r[:NPC] = recip_dst[c * NPC : (c + 1) * NPC]
        recip_t[c] = r.reshape(TPC, P).T

    idx16 = np.zeros((NCORES, 16, ncols * 8), np.int16)
    callslot = gcol * P + prow - colbase[go, qo] * P
    callb = colbase[go, qo] * 8
    idx16[co, callslot % 16, callb + callslot // 16] = qrow[order].astype(
        np.int16
    )
    idx16 = np.ascontiguousarray(np.tile(idx16, (1, 8, 1)))

    inc = [[[] for _ in range(TG)] for _ in range(NG)]
    for gg in range(NG):
        for t in range(TG):
            for qq in range(NQ):
                c = cap[gg, qq, t]
                if c == 0:
                    continue
                s0 = scum[gg, qq, t]
                for cc in range(s0 // P, -(-(s0 + c) // P)):
                    inc[gg][t].append(int(colbase[gg, qq] + cc))
    return edst, recip_t, idx16, ncols, calls, inc


def _build(ncols, calls, inc, skip_bias):
    from concourse import bacc, bass, mybir, tile

    bf16 = mybir.dt.bfloat16
    f16 = mybir.dt.float16
    f32 = mybir.dt.float32
    i16 = mybir.dt.int16
    EQ = mybir.AluOpType.is_equal

    nc = bacc.Bacc(
        "TRN2", target_bir_lowering=False, debug=False, num_devices=NCORES,
        num_swdge_queues=4,
    )

    # x table: one tensor per half, each [2*QROWS, P]; quarter views inside
    xh = [
        nc.declare_dram_parameter(f"xh{i}", [2 * QROWS, P], bf16, isOutput=False)
        for i in range(2)
    ]
    xown = nc.declare_dram_parameter("xown", [NPC_PAD, P], bf16, isOutput=False)
    idx_d = nc.declare_dram_parameter("idx16", [P, ncols * 8], i16, isOutput=False)
    edst_d = nc.declare_dram_parameter("edst", [P, ncols], f32, isOutput=False)
    recp_d = nc.declare_dram_parameter("recp", [P, TPC], f32, isOutput=False)
    iota_d = nc.declare_dram_parameter("iotat", [P, TG * P], f16, isOutput=False)
    wl1_d = nc.declare_dram_parameter("wl1", [P, P], bf16, isOutput=False)
    wr1_d = nc.declare_dram_parameter("wr1", [P, P], bf16, isOutput=False)
    wl2_d = nc.declare_dram_parameter("wl2", [P, P], bf16, isOutput=False)
    wr2_d = nc.declare_dram_parameter("wr2", [P, P], bf16, isOutput=False)
    b1_d = nc.declare_dram_parameter("b1", [1, P], bf16, isOutput=False)
    b2_d = nc.declare_dram_parameter("b2", [1, P], bf16, isOutput=False)
    ones_d = nc.declare_dram_parameter("ones1", [1, P], bf16, isOutput=False)
    out_d = nc.declare_dram_parameter("out", [NPC_PAD, P], f32, isOutput=True)

    maxcallcols = max(c[3] for c in calls)

    with tile.TileContext(nc) as tc:
        with (
            tc.tile_pool(name="const", bufs=1) as cpool,
            tc.tile_pool(name="gath", bufs=4) as gpool,
            tc.tile_pool(name="xot", bufs=3) as tpool,
            tc.tile_pool(name="work", bufs=16) as wpool,
            tc.tile_pool(name="aggp", bufs=6) as apool,
            tc.tile_pool(name="scl", bufs=6) as spool,
            tc.tile_pool(name="outp", bufs=4) as opool,
            tc.tile_pool(name="psacc", bufs=4, space="PSUM") as ps_acc,
            tc.tile_pool(name="psh1", bufs=2, space="PSUM") as ps_h1,
            tc.tile_pool(name="psh2", bufs=2, space="PSUM") as ps_h2,
            tc.tile_pool(name="dram", bufs=1, space="DRAM") as dpool,
        ):
            def cload(dram_ap, shape, dtype, name):
                t = cpool.tile(shape, dtype, name=name)
                nc.sync.dma_start(out=t[:], in_=dram_ap)
                return t

            wl1 = cload(wl1_d[:], [P, P], bf16, "wl1")
            wr1 = cload(wr1_d[:], [P, P], bf16, "wr1")
            wl2 = cload(wl2_d[:], [P, P], bf16, "wl2")
            wr2 = cload(wr2_d[:], [P, P], bf16, "wr2")
            b1 = cload(b1_d[:], [1, P], bf16, "b1")
            b2 = cload(b2_d[:], [1, P], bf16, "b2")
            ones1 = cload(ones_d[:], [1, P], bf16, "ones1")
            iotat = cload(iota_d[:], [P, TG * P], f16, "iotat")
            edst = cload(edst_d[:], [P, ncols], f32, "edst")
            recp = cload(recp_d[:], [P, TPC], f32, "recp")
            idx_sb = cload(idx_d[:], [P, ncols * 8], i16, "idx16")

            h_bounce = dpool.tile([NPC_PAD, P], bf16, name="h_bounce")
            h_half = [
                dpool.tile([2 * QROWS, P], bf16, name=f"h_half{i}",
                           addr_space="Shared")
                for i in range(2)
            ]

            def layer(qtab, selftab, dst_dram, wl, wr, brow, relu):
                pend = []

                def flush(n):
                    while len(pend) > n:
                        pend.pop(0)()

                for b in range(NB):
                    gs = [gg for gg in range(NG) if gg // GB == b]
                    btiles = {}
                    bbase = {}
                    for (bb, qq, c0, ccols) in calls:
                        if bb != b:
                            continue
                        gt = gpool.tile([P, maxcallcols, P], bf16, tag=f"g{qq}")
                        nc.gpsimd.dma_gather(
                            gt[:, :ccols, :],
                            qtab[qq],
                            idx_sb[:, c0 * 8 : (c0 + ccols) * 8],
                            ccols * P,
                            ccols * P,
                            P,
                            single_packet=False,
                            queue_num=qq,
                        )
                        btiles[qq] = gt
                        bbase[qq] = c0
                    for gg in gs:
                        xot = tpool.tile([P, TG * P], bf16, tag="xot")
                        nc.sync.dma_start_transpose(
                            xot[:], selftab[gg * TG * P : (gg + 1) * TG * P, :]
                        )
                        for t in range(TG):
                            cols = inc[gg][t]
                            acc = None
                            if cols:
                                acc = ps_acc.tile([P, P], f32, tag="acc")
                                for ci, gc in enumerate(cols):
                                    for (bb, qq, c0, ccols) in calls:
                                        if bb == b and c0 <= gc < c0 + ccols:
                                            break
                                    ind = wpool.tile([P, P], bf16, tag="ind")
                                    nc.vector.tensor_scalar(
                                        out=ind[:],
                                        in0=iotat[:, t * P : (t + 1) * P],
                                        scalar1=edst[:, gc : gc + 1],
                                        scalar2=None,
                                        op0=EQ,
                                    )
                                    nc.tensor.matmul(
                                        out=acc[:],
                                        lhsT=btiles[qq][:, gc - bbase[qq], :],
                                        rhs=ind[:],
                                        start=(ci == 0),
                                        stop=(ci == len(cols) - 1),
                                    )

                            def hphase(gg=gg, t=t, acc=acc, xot=xot):
                                tg_glob = gg * TG + t
                                h2 = ps_h2.tile([P, P], f32, tag="h2")
                                if not skip_bias:
                                    nc.tensor.matmul(
                                        out=h2[:], lhsT=ones1[:], rhs=brow[:],
                                        start=True, stop=False,
                                    )
                                nc.tensor.matmul(
                                    out=h2[:],
                                    lhsT=xot[:, t * P : (t + 1) * P],
                                    rhs=wr[:],
                                    start=skip_bias, stop=True,
                                )
                                if acc is not None:
                                    aggT = apool.tile([P, P], bf16, tag="aggT")
                                    nc.scalar.activation(
                                        out=aggT[:], in_=acc[:],
                                        func=mybir.ActivationFunctionType.Copy,
                                    )
                                    h1 = ps_h1.tile([P, P], f32, tag="h1")
                                    nc.tensor.matmul(
                                        out=h1[:], lhsT=aggT[:], rhs=wl[:],
                                        start=True, stop=True,
                                    )
                                    # mean normalization: per-partition (dst)
                                    # scale on the Scalar engine
                                    t1 = spool.tile([P, P], f32, tag="t1")
                                    nc.scalar.activation(
                                        out=t1[:], in_=h1[:],
                                        func=mybir.ActivationFunctionType.Copy,
                                        scale=recp[:, tg_glob : tg_glob + 1],
                                    )
                                    if relu:
                                        tsum = spool.tile([P, P], f32, tag="ts")
                                        nc.vector.tensor_add(
                                            out=tsum[:], in0=t1[:], in1=h2[:]
                                        )
                                        hsb = opool.tile([P, P], bf16, tag="hs")
                                        nc.scalar.activation(
                                            out=hsb[:], in_=tsum[:],
                                            func=mybir.ActivationFunctionType.Relu,
                                        )
                                    else:
                                        hsb = opool.tile([P, P], f32, tag="hs32")
                                        nc.vector.tensor_add(
                                            out=hsb[:], in0=t1[:], in1=h2[:]
                                        )
                                else:
                                    if relu:
                                        hsb = opool.tile([P, P], bf16, tag="hs")
                                        nc.scalar.activation(
                                            out=hsb[:], in_=h2[:],
                                            func=mybir.ActivationFunctionType.Relu,
                                        )
                                    else:
                                        hsb = opool.tile([P, P], f32, tag="hs32")
                                        nc.vector.tensor_copy(
                                            out=hsb[:], in_=h2[:]
                                        )
                                nc.sync.dma_start(
                                    out=dst_dram[
                                        tg_glob * P : (tg_glob + 1) * P, :
                                    ],
                                    in_=hsb[:],
                                )

                            pend.append(hphase)
                            flush(2)
                flush(0)

            xq_aps = [
                xh[0][0:QROWS, :], xh[0][QROWS : 2 * QROWS, :],
                xh[1][0:QROWS, :], xh[1][QROWS : 2 * QROWS, :],
            ]
            layer(xq_aps, xown, h_bounce, wl1, wr1, b1, relu=True)
            for i in range(2):
                nc.gpsimd.collective_compute(
                    "AllGather",
                    mybir.AluOpType.bypass,
                    replica_groups=[list(range(NCORES))],
                    ins=[h_bounce[i * HROWS : (i + 1) * HROWS, :]],
                    outs=[h_half[i][:]],
                )
            hq_aps = [
                h_half[0][0:QROWS, :], h_half[0][QROWS : 2 * QROWS, :],
                h_half[1][0:QROWS, :], h_half[1][QROWS : 2 * QROWS, :],
            ]
            layer(hq_aps, h_bounce, out_d, wl2, wr2, b2, relu=False)

    return nc


def run(x, edge_index, W_l1, b_l1, W_r1, W_l2, b_l2, W_r2, trace=False):
    import ml_dtypes

    bf = ml_dtypes.bfloat16
    n_nodes = x.shape[0]
    assert n_nodes == NCORES * NPC

    edst, recip_t, idx16, ncols, calls, inc = _prep(np.asarray(edge_index))

    x = np.asarray(x, np.float32)
    # per-core padded slices, then relayout to [half, core, HROWS]
    xp = np.zeros((NCORES, NPC_PAD, P), bf)
    for c in range(NCORES):
        xp[c, :NPC] = x[c * NPC : (c + 1) * NPC]
    x_pad = np.ascontiguousarray(
        xp.reshape(NCORES, 2, HROWS, P).transpose(1, 0, 2, 3)
    ).reshape(NALL, P)

    iotat = np.tile(np.arange(TG * P, dtype=np.float16), (P, 1))
    skip_bias = not (np.any(np.asarray(b_l1)) or np.any(np.asarray(b_l2)))
    common = {
        "xh0": np.ascontiguousarray(x_pad[: NALL // 2]),
        "xh1": np.ascontiguousarray(x_pad[NALL // 2 :]),
        "wl1": np.asarray(W_l1, bf),
        "wr1": np.asarray(W_r1, bf),
        "wl2": np.asarray(W_l2, bf),
        "wr2": np.asarray(W_r2, bf),
        "b1": np.asarray(b_l1, bf).reshape(1, P),
        "b2": np.asarray(b_l2, bf).reshape(1, P),
        "ones1": np.ones((1, P), bf),
        "iotat": np.ascontiguousarray(iotat),
    }
    in_maps = []
    for c in range(NCORES):
        m = dict(common)
        m["xown"] = np.ascontiguousarray(xp[c])
        m["idx16"] = idx16[c]
        m["edst"] = np.ascontiguousarray(edst[c])
        m["recp"] = np.ascontiguousarray(recip_t[c])
        in_maps.append(m)

    nc = _build(ncols, calls, inc, skip_bias)
    nc.finalize()

    from concourse.bass_utils import run_bass_kernel_spmd

    res = run_bass_kernel_spmd(nc, in_maps, list(range(NCORES)), trace=trace)
    out = np.empty((n_nodes, P), np.float32)
    for c in range(NCORES):
        out[c * NPC : (c + 1) * NPC] = res.results[c]["out"][:NPC]
    return out, res


def kernel(x, edge_index, W_l1, b_l1, W_r1, W_l2, b_l2, W_r2):
    out, _ = run(x, edge_index, W_l1, b_l1, W_r1, W_l2, b_l2, W_r2)
    return out
